# revision 1
# baseline (speedup 1.0000x reference)
"""Trainium2 Bass kernel for nn_DeepSetAttentionModel (segment_reduce).

Strategy (pure data parallel, 8 NeuronCores):
- Host sorts the 64 set rows by length (desc) and assigns rank k to
  core k%8, slot k//8. All cores run ONE SPMD program whose per-slot
  token counts are the max length within the slot's 8 rows, so the
  program is identical across cores while skipping most padding work.
- Per core, each row is processed in 128-token chunks. The phi/psi MLPs
  run feature-major on the tensor engine in bf16 (f32 PSUM accumulate);
  attention is folded: preattn = Vx.T @ x + agg @ Vagg per head, with
  Vx/Vagg = per-head W_k @ W_q computed once on device at setup. The
  demo token is appended as the last chunk. Invalid tokens get -1e9
  before softmax so exp() underflows to 0 and they drop out of every
  segment reduction (mean, softmax, weighted sum).
- Softmax/aggregation arithmetic stays f32.
"""
import numpy as np

B, T = 64, 4096
CH = 128
NPOS, V, NMOD = 16, 1, 15
PHI_IN = 32
DP, H = 64, 4
MAXTS = 100.0
NCORES = 8
NSLOTS = B // NCORES

BF16_W = ["demo_w1", "demo_w2", "phi_w1", "phi_w2", "phi_w3",
          "psi_w1", "psi_w2", "psi_w3", "rho_w1"]


def _host_constants():
    ts = MAXTS ** np.linspace(0.0, 1.0, NPOS // 2).astype(np.float32)
    twopi = 2.0 * np.pi
    inv_ts2 = (np.concatenate([1.0 / ts, 1.0 / ts]) / twopi).astype(np.float32)  # [16] turns
    phase = np.array([0.0] * 8 + [0.25] * 8, np.float32)                 # [16] turns
    itp = np.concatenate([inv_ts2, phase])[None, :].repeat(128, 0)      # [128,32]
    iota15 = np.arange(NMOD, dtype=np.float32)[None, :].repeat(128, 0)  # [128,15]
    iota_tok = (np.arange(128, dtype=np.float32)[:, None]
                + 128.0 * np.arange(T // CH, dtype=np.float32)[None, :])  # [128,32]
    iota_hm = np.arange(T, dtype=np.float32)[None, :].repeat(H, 0)      # [4,4096]
    ident = np.eye(128, dtype=np.float32)
    return itp, iota15, iota_tok, iota_hm, ident


def _build_nc(Cs, tile_mod, bass, mybir):
    """Build the SPMD program for per-slot chunk counts Cs (len 8)."""
    f32 = mybir.dt.float32
    bf16 = mybir.dt.bfloat16
    i32 = mybir.dt.int32
    Alu = mybir.AluOpType
    Act = mybir.ActivationFunctionType
    Cmax = max(Cs)

    nc = bass.Bass()
    dt_in = {}

    def din(name, shape, dtype=f32):
        dt_in[name] = nc.dram_tensor(name, list(shape), dtype, kind="ExternalInput")
        return dt_in[name]

    # per-core inputs (slot-ordered rows)
    d_times = din("times_r", [NSLOTS, T])
    d_vals = din("values_r", [NSLOTS, T])
    d_meas = din("meas_r", [NSLOTS, T], i32)
    d_demo = din("demo_r", [NSLOTS, 8], bf16)
    din("lens_bcast", [128, NSLOTS])
    # constants
    din("itp", [128, 32])
    din("iota15", [128, NMOD])
    din("iota_tok", [128, T // CH])
    din("iota_hm", [H, T])
    din("ident", [128, 128], bf16)
    din("identf", [128, 128])
    din("WqT_s", [DP, H])
    # weights
    for nm, shp in [
        ("demo_w1", [8, 128]), ("demo_b1", [128]), ("demo_w2", [128, 32]), ("demo_b2", [32]),
        ("phi_w1", [32, 128]), ("phi_b1", [128]), ("phi_w2", [128, 128]), ("phi_b2", [128]),
        ("phi_w3", [128, 128]), ("phi_b3", [128]),
        ("psi_w1", [32, 128]), ("psi_b1", [128]), ("psi_w2", [128, 128]), ("psi_b2", [128]),
        ("psi_w3", [128, 128]), ("psi_b3", [128]),
        ("rho_attn_w", [128, 128]), ("rho_attn_b", [128]),
        ("W_k", [PHI_IN + 128, DP * H]),
        ("rho_w1", [512, 128]), ("rho_b1", [128]),
        ("rho_w2", [128, 128]), ("rho_b2", [128]),
        ("rho_w3", [128, 1]), ("rho_b3", [1]),
    ]:
        din(nm, shp, bf16 if nm in BF16_W else f32)
    d_out = nc.dram_tensor("out", [NSLOTS, 1], f32, kind="ExternalOutput")

    from contextlib import ExitStack
    with tile_mod.TileContext(nc) as tc, ExitStack() as stack:
        cp = stack.enter_context(tc.tile_pool(name="const", bufs=1))
        sp = stack.enter_context(tc.tile_pool(name="sbuf", bufs=1))
        pp = stack.enter_context(tc.tile_pool(name="psum", bufs=1, space="PSUM"))

        def ctile(shape, dtype=f32, name="ct"):
            return cp.tile(shape, dtype, tag=name, name=name)

        # ---- load constants / weights ----
        def load(name, shape, dtype=f32, ap=None):
            t = ctile(shape, dtype, name=name)
            nc.sync.dma_start(out=t[:], in_=ap if ap is not None else dt_in[name][:])
            return t

        itp = load("itp", [128, 32])
        iota15 = load("iota15", [128, NMOD])
        iotatok = load("iota_tok", [128, T // CH])
        iotahm = load("iota_hm", [H, T])
        ident = load("ident", [128, 128], bf16)
        identf = load("identf", [128, 128])
        wqt = load("WqT_s", [DP, H])
        lensb = load("lens_bcast", [128, NSLOTS])
        demoT = load("demoT", [8, NSLOTS], bf16, ap=d_demo[:].rearrange("r f -> f r"))

        wt = {}
        for nm, shp in [("demo_w1", [8, 128]), ("demo_w2", [128, 32]),
                        ("phi_w1", [32, 128]), ("phi_w2", [128, 128]), ("phi_w3", [128, 128]),
                        ("psi_w1", [32, 128]), ("psi_w2", [128, 128]), ("psi_w3", [128, 128]),
                        ("rho_attn_w", [128, 128]), ("rho_w2", [128, 128]), ("rho_w3", [128, 1])]:
            wt[nm] = load(nm, shp, bf16 if nm in BF16_W else f32)
        for nm, n in [("demo_b1", 128), ("demo_b2", 32), ("phi_b1", 128), ("phi_b2", 128),
                      ("phi_b3", 128), ("psi_b1", 128), ("psi_b2", 128), ("psi_b3", 128),
                      ("rho_attn_b", 128), ("rho_b1", 128), ("rho_b2", 128), ("rho_b3", 1)]:
            wt[nm] = load(nm, [n, 1], ap=dt_in[nm][:].unsqueeze(1))
        wt["rho_w1"] = load("rho_w1", [128, 4, 128], bf16,
                            ap=dt_in["rho_w1"][:].rearrange("(h k) m -> k h m", h=4))
        wkx = load("wkx", [PHI_IN, DP * H], ap=dt_in["W_k"][0:PHI_IN, :])
        wkagg = load("wkagg", [128, DP * H], ap=dt_in["W_k"][PHI_IN:, :])

        # ---- setup: derived small tensors ----
        lp1 = ctile([128, NSLOTS], name="lp1")
        nc.vector.tensor_scalar(lp1[:], lensb[:], 1.0, None, Alu.add)
        recipL1 = ctile([128, NSLOTS], name="recipL1")
        nc.vector.reciprocal(recipL1[:], lp1[:])

        # Vx [32,4] (bf16, feeds preattn), Vagg [128,4] (f32, feeds const matmul)
        ps_vx = pp.tile([128, 4], f32, tag="psisum", name="ps_vx")
        ps_vagg = pp.tile([128, 4], f32, tag="feat", name="ps_vagg")
        Vx = ctile([PHI_IN, H], bf16, name="Vx")
        Vagg = ctile([128, H], name="Vagg")
        for h in range(H):
            pxt = pp.tile([128, 512], f32, tag="mlp", bufs=2, name="pxt")
            nc.tensor.transpose(pxt[0:DP, 0:PHI_IN], wkx[:, h * DP:(h + 1) * DP],
                                identf[0:PHI_IN, 0:PHI_IN])
            sxt = sp.tile([DP, 128], f32, tag="sxt", bufs=2, name="sxt")
            nc.vector.tensor_copy(sxt[:, 0:PHI_IN], pxt[0:DP, 0:PHI_IN])
            nc.tensor.matmul(ps_vx[0:PHI_IN, h:h + 1], sxt[:, 0:PHI_IN], wqt[:, h:h + 1])
            pxt2 = pp.tile([128, 512], f32, tag="mlp", bufs=2, name="pxt2")
            nc.tensor.transpose(pxt2[0:DP, 0:128], wkagg[:, h * DP:(h + 1) * DP], identf[:])
            sxt2 = sp.tile([DP, 128], f32, tag="sxt", bufs=2, name="sxt2")
            nc.vector.tensor_copy(sxt2[:], pxt2[0:DP, 0:128])
            nc.tensor.matmul(ps_vagg[:, h:h + 1], sxt2[:], wqt[:, h:h + 1])
        nc.vector.tensor_copy(Vx[:], ps_vx[0:PHI_IN, :])
        nc.vector.tensor_copy(Vagg[:], ps_vagg[:])

        # ---- demo encoder for all 8 slots ----
        ps_d = pp.tile([128, 512], f32, tag="mlp", bufs=2, name="ps_d")
        nc.tensor.matmul(ps_d[:, 0:NSLOTS], wt["demo_w1"][:], demoT[:])
        dh1 = ctile([128, NSLOTS], bf16, name="dh1")
        nc.scalar.activation(dh1[:], ps_d[:, 0:NSLOTS], Act.Relu, bias=wt["demo_b1"][:])
        ps_d2 = pp.tile([128, 512], f32, tag="mlp", bufs=2, name="ps_d2")
        nc.tensor.matmul(ps_d2[0:PHI_IN, 0:NSLOTS], wt["demo_w2"][:], dh1[:])
        demo_encT = ctile([PHI_IN + 1, NSLOTS], bf16, name="demo_encT")
        nc.scalar.activation(demo_encT[0:PHI_IN, :], ps_d2[0:PHI_IN, 0:NSLOTS],
                             Act.Identity, bias=wt["demo_b2"][:])
        nc.gpsimd.memset(demo_encT[PHI_IN:PHI_IN + 1, :], 1.0)

        # demo tokens through phi and psi (feature-major, 8 cols)
        def mlp3(prefix, rhs, ncols, out_dtype):
            cur = rhs
            for li, (w, b, din_) in enumerate([
                    (wt[prefix + "_w1"], wt[prefix + "_b1"], PHI_IN),
                    (wt[prefix + "_w2"], wt[prefix + "_b2"], 128),
                    (wt[prefix + "_w3"], wt[prefix + "_b3"], 128)]):
                ps = pp.tile([128, 512], f32, tag="mlp", bufs=2, name=f"ps_{prefix}{li}")
                nc.tensor.matmul(ps[:, 0:ncols], w[:], cur[0:din_, 0:ncols])
                dt_ = out_dtype if li == 2 else bf16
                nxt = sp.tile([128, NSLOTS], dt_, tag=f"demo_{prefix}{li}", name=f"dm_{prefix}{li}")
                nc.scalar.activation(nxt[:, 0:ncols], ps[:, 0:ncols], Act.Relu, bias=b[:])
                cur = nxt
            return cur

        denc_fm = mlp3("phi", demo_encT, NSLOTS, bf16)    # [128, 8] bf16
        psi_demo = mlp3("psi", demo_encT, NSLOTS, f32)    # [128, 8] f32
        ps_dt = pp.tile([128, 512], bf16, tag="xpose", bufs=2, name="ps_dt")
        nc.tensor.transpose(ps_dt[0:NSLOTS, 0:128], denc_fm[:, 0:NSLOTS], ident[:])
        enc_demo_tok = ctile([NSLOTS, 128], bf16, name="enc_demo_tok")
        nc.vector.tensor_copy(enc_demo_tok[:], ps_dt[0:NSLOTS, 0:128])

        feat_all = sp.tile([128, NSLOTS, H], bf16, tag="feat_all", name="feat_all")

        # ---- per-row processing ----
        for r in range(NSLOTS):
            C = Cs[r]
            Tp = C * CH
            Text = (C + 1) * CH
            NG = (C + 3) // 4  # groups of up to 4 chunks

            times_sb = sp.tile([128, C], f32, tag="times", bufs=2, name="times_sb")
            nc.gpsimd.dma_start(out=times_sb[:], in_=d_times[r, 0:Tp].rearrange("(c p) -> p c", p=128))
            vals_sb = sp.tile([128, C], f32, tag="vals", bufs=2, name="vals_sb")
            nc.gpsimd.dma_start(out=vals_sb[:], in_=d_vals[r, 0:Tp].rearrange("(c p) -> p c", p=128))
            meas_sb = sp.tile([128, C], i32, tag="meas", bufs=2, name="meas_sb")
            nc.gpsimd.dma_start(out=meas_sb[:], in_=d_meas[r, 0:Tp].rearrange("(c p) -> p c", p=128))
            measf = sp.tile([128, C], f32, tag="measf", bufs=2, name="measf")
            nc.vector.tensor_copy(measf[:], meas_sb[:])

            mask_tok = sp.tile([128, C], bf16, tag="mask_tok", bufs=2, name="mask_tok")
            nc.vector.tensor_scalar(mask_tok[:], iotatok[:, 0:C], lensb[:, r:r + 1], None, Alu.is_lt)

            # featurize: x_tok [128, C, 33] bf16; angle math in f32 scratch
            xtok = sp.tile([128, Cmax, 33], bf16, tag="xtok", bufs=2, name="xtok")
            ang = sp.tile([128, Cmax, 16], f32, tag="ang", bufs=2, name="ang")
            tri = sp.tile([128, Cmax, 16], i32, tag="tri", bufs=2, name="tri")
            trf = sp.tile([128, Cmax, 16], f32, tag="trf", bufs=2, name="trf")
            nc.vector.tensor_tensor(
                out=ang[:, 0:C, :],
                in0=times_sb[:].unsqueeze(2).to_broadcast([128, C, 16]),
                in1=itp[:, 0:16].unsqueeze(1).to_broadcast([128, C, 16]),
                op=Alu.mult)
            nc.vector.tensor_tensor(
                out=ang[:, 0:C, :],
                in0=ang[:, 0:C, :],
                in1=itp[:, 16:32].unsqueeze(1).to_broadcast([128, C, 16]),
                op=Alu.add)
            nc.vector.tensor_copy(tri[:, 0:C, :], ang[:, 0:C, :])
            nc.vector.tensor_copy(trf[:, 0:C, :], tri[:, 0:C, :])
            nc.vector.tensor_tensor(out=ang[:, 0:C, :], in0=ang[:, 0:C, :],
                                    in1=trf[:, 0:C, :], op=Alu.subtract)
            nc.vector.tensor_scalar(trf[:, 0:C, :], ang[:, 0:C, :], 0.5, None, Alu.is_gt)
            nc.vector.tensor_tensor(out=ang[:, 0:C, :], in0=ang[:, 0:C, :],
                                    in1=trf[:, 0:C, :], op=Alu.subtract)
            nc.scalar.activation(xtok[:, 0:C, 0:16], ang[:, 0:C, :], Act.Sin,
                                 scale=float(2.0 * np.pi))
            nc.vector.tensor_copy(xtok[:, 0:C, 16:17], vals_sb[:].unsqueeze(2))
            nc.vector.tensor_tensor(
                out=xtok[:, 0:C, 17:32],
                in0=measf[:].unsqueeze(2).to_broadcast([128, C, NMOD]),
                in1=iota15[:].unsqueeze(1).to_broadcast([128, C, NMOD]),
                op=Alu.is_equal)
            nc.gpsimd.memset(xtok[:, 0:C, 32:33], 1.0)

            # transpose x -> xT [33, Tp] feature-major (bf16)
            xT = sp.tile([PHI_IN + 1, Cmax * CH], bf16, tag="xT", bufs=2, name="xT")
            for g in range(NG):
                c0 = g * 4
                nch = min(4, C - c0)
                pxp = pp.tile([128, 512], bf16, tag="xpose", bufs=2, name="pxp")
                for j in range(nch):
                    nc.tensor.transpose(pxp[0:PHI_IN + 1, j * CH:(j + 1) * CH],
                                        xtok[:, c0 + j, 0:PHI_IN + 1], ident[:])
                nc.scalar.copy(xT[:, c0 * CH:(c0 + nch) * CH], pxp[0:PHI_IN + 1, 0:nch * CH])

            # phi MLP feature-major -> enc groups -> transpose -> enc_tok (bf16)
            enc_tok = sp.tile([128, Cmax + 1, 128], bf16, tag="enc_tok", bufs=2, name="enc_tok")
            psum_psi = pp.tile([128, 4], f32, tag="psisum", name="psum_psi")

            def layer(w, b, rhs_tile, rhs_rows, out_tile, g, N, relu_engine):
                ps = pp.tile([128, 512], f32, tag="mlp", bufs=2, name="ps_mlp")
                nc.tensor.matmul(ps[:, 0:N], w[:], rhs_tile[0:rhs_rows, g * 512:g * 512 + N])
                if relu_engine == "act":
                    nc.scalar.activation(out_tile[:, g * 512:g * 512 + N], ps[:, 0:N],
                                         Act.Relu, bias=b[:])
                else:
                    nc.vector.tensor_scalar(out_tile[:, g * 512:g * 512 + N], ps[:, 0:N],
                                            b[:], 0.0, Alu.add, Alu.max)

            h1 = sp.tile([128, Cmax * CH], bf16, tag="h_a", bufs=2, name="h1")
            h2 = sp.tile([128, Cmax * CH], bf16, tag="h_b", bufs=2, name="h2")
            for g in range(NG):
                N = min(512, Tp - g * 512)
                layer(wt["phi_w1"], wt["phi_b1"], xT, PHI_IN, h1, g, N, "act")
            for g in range(NG):
                N = min(512, Tp - g * 512)
                layer(wt["phi_w2"], wt["phi_b2"], h1, 128, h2, g, N, "act")
            for g in range(NG):
                N = min(512, Tp - g * 512)
                c0 = g * 4
                nch = min(4, C - c0)
                fm = sp.tile([128, 512], bf16, tag="fm", bufs=2, name="enc_fm")
                ps = pp.tile([128, 512], f32, tag="mlp", bufs=2, name="ps_phi3")
                nc.tensor.matmul(ps[:, 0:N], wt["phi_w3"][:], h2[:, g * 512:g * 512 + N])
                nc.scalar.activation(fm[:, 0:N], ps[:, 0:N], Act.Relu, bias=wt["phi_b3"][:])
                nc.sync.dma_start_transpose(out=enc_tok[:, c0:c0 + nch, :], in_=fm[:, 0:N])

            # psi MLP -> psi_tok groups -> masked segment sum via matmul
            p1 = sp.tile([128, Cmax * CH], bf16, tag="h_a", bufs=2, name="p1")
            p2 = sp.tile([128, Cmax * CH], bf16, tag="h_b", bufs=2, name="p2")
            for g in range(NG):
                N = min(512, Tp - g * 512)
                layer(wt["psi_w1"], wt["psi_b1"], xT, PHI_IN, p1, g, N, "vec")
            for g in range(NG):
                N = min(512, Tp - g * 512)
                layer(wt["psi_w2"], wt["psi_b2"], p1, 128, p2, g, N, "vec")
            for g in range(NG):
                N = min(512, Tp - g * 512)
                c0 = g * 4
                nch = min(4, C - c0)
                fmp = sp.tile([128, 512], bf16, tag="fm", bufs=2, name="psi_fm")
                ps = pp.tile([128, 512], f32, tag="mlp", bufs=2, name="ps_psi3")
                nc.tensor.matmul(ps[:, 0:N], wt["psi_w3"][:], p2[:, g * 512:g * 512 + N])
                nc.vector.tensor_scalar(fmp[:, 0:N], ps[:, 0:N], wt["psi_b3"][:], 0.0, Alu.add, Alu.max)
                ptok = sp.tile([128, 4, 128], bf16, tag="psi_tok", bufs=2, name="psi_tok")
                nc.sync.dma_start_transpose(out=ptok[:, 0:nch, :], in_=fmp[:, 0:N])
                for j in range(nch):
                    c = c0 + j
                    nc.tensor.matmul(psum_psi[:, 0:1], ptok[:, j, :], mask_tok[:, c:c + 1],
                                     start=(c == 0), stop=(c == C - 1))

            # demo enc into last chunk
            nc.gpsimd.memset(enc_tok[:, C, :], 0.0)
            nc.sync.dma_start(out=enc_tok[0:1, C, :], in_=enc_demo_tok[r:r + 1, :])

            # agg chain (f32)
            agg_in = sp.tile([128, 1], f32, tag="agg_in", bufs=2, name="agg_in")
            nc.vector.tensor_scalar(agg_in[:], psum_psi[:, 0:1], psi_demo[:, r:r + 1],
                                    recipL1[:, r:r + 1], Alu.add, Alu.mult)
            ps_agg = pp.tile([128, 4], f32, tag="psisum", name="ps_agg")
            nc.tensor.matmul(ps_agg[:, 0:1], wt["rho_attn_w"][:], agg_in[:])
            agg2 = sp.tile([128, 1], f32, tag="agg2", bufs=2, name="agg2")
            nc.scalar.activation(agg2[:], ps_agg[:, 0:1], Act.Relu, bias=wt["rho_attn_b"][:])
            ps_cr = pp.tile([4, 512], f32, tag="pre", bufs=2, name="ps_cr")
            nc.tensor.matmul(ps_cr[0:1, 0:H], agg2[:], Vagg[:])
            vxe = sp.tile([PHI_IN + 1, H], bf16, tag="vxe", bufs=2, name="vxe")
            nc.vector.tensor_copy(vxe[0:PHI_IN, :], Vx[:])
            nc.vector.tensor_copy(vxe[PHI_IN:PHI_IN + 1, :], ps_cr[0:1, 0:H])

            # preattn + mask (f32 psum, f32 softmax)
            pre_sb = sp.tile([H, (Cmax + 1) * CH], f32, tag="pre_sb", bufs=2, name="pre_sb")
            maskc = sp.tile([H, Cmax * CH], bf16, tag="maskc", bufs=2, name="maskc")
            nc.vector.tensor_scalar(maskc[:, 0:Tp], iotahm[:, 0:Tp], lensb[0:H, r:r + 1],
                                    -1e9, Alu.is_ge, Alu.mult)
            for g in range(NG):
                N = min(512, Tp - g * 512)
                ps = pp.tile([4, 512], f32, tag="pre", bufs=2, name="ps_pre")
                nc.tensor.matmul(ps[:, 0:N], vxe[:], xT[:, g * 512:g * 512 + N])
                nc.vector.tensor_tensor(out=pre_sb[:, g * 512:g * 512 + N], in0=ps[:, 0:N],
                                        in1=maskc[:, g * 512:g * 512 + N], op=Alu.add)
            ps_pd = pp.tile([4, 512], f32, tag="pre", bufs=2, name="ps_pd")
            nc.tensor.matmul(ps_pd[:, 0:1], vxe[:], demo_encT[:, r:r + 1])
            nc.gpsimd.memset(pre_sb[:, Tp:Text], -1e9)
            nc.vector.tensor_copy(pre_sb[:, Tp:Tp + 1], ps_pd[:, 0:1])

            # softmax (normalized into e_sb, f32)
            negmax = sp.tile([H, 1], f32, tag="negmax", bufs=2, name="negmax")
            nc.vector.tensor_reduce(negmax[:], pre_sb[:, 0:Text], mybir.AxisListType.X,
                                    Alu.max, negate=True)
            e_sb = sp.tile([H, (Cmax + 1) * CH], bf16, tag="e_sb", bufs=2, name="e_sb")
            zt = sp.tile([H, 1], f32, tag="zt", bufs=2, name="zt")
            nc.scalar.activation(e_sb[:, 0:Text], pre_sb[:, 0:Text], Act.Exp,
                                 bias=negmax[:], accum_out=zt[:])
            rz = sp.tile([H, 1], f32, tag="rz", bufs=2, name="rz")
            nc.vector.reciprocal(rz[:], zt[:])

            # transpose (unnormalized) attn to token-major, head_agg in [4,128]
            ps_et = pp.tile([128, (Cmax + 1) * H], bf16, tag="xpose", bufs=2, name="ps_et")
            for c in range(C + 1):
                nc.tensor.transpose(ps_et[:, c * H:(c + 1) * H], e_sb[:, c * CH:(c + 1) * CH],
                                    ident[0:H, 0:H])
            e_tok = sp.tile([128, Cmax + 1, H], bf16, tag="e_tok", bufs=2, name="e_tok")
            nc.vector.tensor_copy(e_tok[:, 0:C + 1, :], ps_et[:, 0:(C + 1) * H])
            ps_hh = pp.tile([4, 512], f32, tag="feat", name="ps_hh")
            for c in range(C + 1):
                nc.tensor.matmul(ps_hh[:, 0:128], e_tok[:, c, :], enc_tok[:, c, :],
                                 start=(c == 0), stop=(c == C))
            hh_sb = sp.tile([H, 128], bf16, tag="hh_sb", bufs=2, name="hh_sb")
            nc.vector.tensor_scalar(hh_sb[:], ps_hh[:, 0:128], rz[:], None, Alu.mult)
            ps_tr = pp.tile([128, 4], bf16, tag="feat", name="ps_tr")
            nc.tensor.transpose(ps_tr[:], hh_sb[:], ident[0:H, 0:H])
            nc.vector.tensor_copy(feat_all[:, r, :], ps_tr[:])

        # ---- rho MLP over all 8 rows (f32) ----
        ps_r1 = pp.tile([128, 512], f32, tag="mlp", bufs=2, name="ps_r1")
        for h in range(H):
            nc.tensor.matmul(ps_r1[:, 0:NSLOTS], wt["rho_w1"][:, h, :], feat_all[:, :, h],
                             start=(h == 0), stop=(h == H - 1))
        r1 = sp.tile([128, NSLOTS], f32, tag="r1", name="r1")
        nc.scalar.activation(r1[:], ps_r1[:, 0:NSLOTS], Act.Relu, bias=wt["rho_b1"][:])
        ps_r2 = pp.tile([128, 512], f32, tag="mlp", bufs=2, name="ps_r2")
        nc.tensor.matmul(ps_r2[:, 0:NSLOTS], wt["rho_w2"][:], r1[:])
        r2 = sp.tile([128, NSLOTS], f32, tag="r2", name="r2")
        nc.scalar.activation(r2[:], ps_r2[:, 0:NSLOTS], Act.Relu, bias=wt["rho_b2"][:])
        ps_r3 = pp.tile([4, 512], f32, tag="pre", bufs=2, name="ps_r3")
        nc.tensor.matmul(ps_r3[0:1, 0:NSLOTS], wt["rho_w3"][:], r2[:])
        res = sp.tile([1, NSLOTS], f32, tag="res", name="res")
        nc.scalar.activation(res[:], ps_r3[0:1, 0:NSLOTS], Act.Sigmoid, bias=wt["rho_b3"][:])
        nc.sync.dma_start(out=d_out[:].rearrange("r one -> one r"), in_=res[:])
    return nc


def _patch_tile_drain(tile_mod, mybir):
    """Walrus in this env rejects >1 sync wait per instruction. Two fixes:
    1) split the Tile tail drain's waits across sequential drains;
    2) a post-pass over the final BIR that moves extra waits of ANY
       instruction onto standalone NoOps inserted just before it."""
    from concourse.vector_clock import ScopedClock
    if getattr(tile_mod.TileContext, "_drain_patched", False):
        return

    def _drain_and_barrier(self, tick_clock, wait_clock):
        nc = self.nc
        drain_inst = nc.sync.drain()
        wait_clock.add_sem_waits(drain_inst.ins, ScopedClock({None: tick_clock.global_clock}))
        si = drain_inst.ins.sync_info
        waits = list(si.on_wait or [])
        if len(waits) > 1:
            si.on_wait = waits[:1]
            for i in range(1, len(waits)):
                extra = nc.sync.drain()
                esi = extra.ins.sync_info
                if esi is None:
                    extra.ins.sync_info = mybir.SyncInfo(on_wait=waits[i:i + 1], on_update=[])
                else:
                    esi.on_wait = waits[i:i + 1]
        nc.all_engine_barrier()
        popped = nc._tile_sem_poison_stack.pop()
        assert popped is self._sem_poison
        nc.clear_and_free_semaphores(list(self.sems.allocated().values()))
        nc.all_engine_barrier()

    tile_mod.TileContext._drain_and_barrier = _drain_and_barrier

    _orig_exit = tile_mod.TileContext.__exit__

    def _exit(self, exc_type, exc_val, exc_tb):
        r = _orig_exit(self, exc_type, exc_val, exc_tb)
        if exc_type is None and getattr(tile_mod.TileContext, "_split_waits", True):
            _split_multi_waits(self.nc, mybir)
        return r

    def _split_multi_waits(nc, mybir):
        n = [0]
        for f in nc.m.functions:
            for bb in f.blocks:
                insts = bb.instructions
                out = []
                for inst in insts:
                    si = inst.sync_info
                    waits = list(si.on_wait) if (si and si.on_wait) else []
                    if len(waits) > 1:
                        for w in waits[:-1]:
                            n[0] += 1
                            nop = mybir.InstNoOp(name=f"I-ws-{n[0]}", ins=[], outs=[])
                            nop.engine = inst.engine
                            nop.sync_info = mybir.SyncInfo(on_wait=[w], on_update=[])
                            out.append(nop)
                        si.on_wait = waits[-1:]
                    out.append(inst)
                if len(out) != len(insts):
                    bb.instructions = out

    tile_mod.TileContext.__exit__ = _exit
    tile_mod.TileContext._drain_patched = True


_CACHE = {}
last_results = None


def _maybe_install_ntff_shim():
    """The image's antenv lacks axon_hooks; register the ctypes NTFF hook so
    run_bass_kernel_spmd(trace=True) can profile."""
    import sys, types
    if "antenv.axon_hooks" in sys.modules:
        return
    try:
        from trn_agent_boot.trn_boot import _ntff_profile_via_ctypes
        hook = _ntff_profile_via_ctypes("/opt/axon/libaxon_pjrt.so")
    except Exception:
        hook = None
    mod = types.ModuleType("antenv.axon_hooks")
    mod.get_axon_ntff_profile_hook = lambda: hook
    sys.modules["antenv.axon_hooks"] = mod


def _to_bf16(a):
    import ml_dtypes
    return np.asarray(a, np.float32).astype(ml_dtypes.bfloat16)


def kernel(**inputs):
    import os
    import concourse.bass as bass
    import concourse.mybir as mybir
    import concourse.tile as tile_mod
    from concourse import bass_utils

    _patch_tile_drain(tile_mod, mybir)

    inp = {k: np.asarray(v) for k, v in inputs.items()}
    times = np.ascontiguousarray(inp["times"].astype(np.float32)[..., 0])    # [B,T]
    values = np.ascontiguousarray(inp["values"].astype(np.float32)[..., 0])  # [B,T]
    meas = np.ascontiguousarray(inp["measurements"].astype(np.int32))        # [B,T]
    lengths = inp["lengths"].astype(np.int64)                                # [B]
    demo = inp["demo"].astype(np.float32)

    order = np.argsort(-lengths, kind="stable")
    Cs = []
    for s in range(NSLOTS):
        ranks = order[s * NCORES:(s + 1) * NCORES]
        Cs.append(int(np.ceil(lengths[ranks].max() / CH)))

    key = tuple(Cs)
    if key not in _CACHE:
        _CACHE[key] = _build_nc(Cs, tile_mod, bass, mybir)
    nc = _CACHE[key]

    itp, iota15, iota_tok, iota_hm, ident = _host_constants()
    wq = inp["W_q"].astype(np.float32) * (1.0 / np.sqrt(DP))
    const_map = {
        "itp": itp, "iota15": iota15, "iota_tok": iota_tok, "iota_hm": iota_hm,
        "ident": _to_bf16(ident), "identf": ident,
        "WqT_s": np.ascontiguousarray(wq.T),
    }
    wnames = ["demo_w1", "demo_b1", "demo_w2", "demo_b2", "phi_w1", "phi_b1", "phi_w2",
              "phi_b2", "phi_w3", "phi_b3", "psi_w1", "psi_b1", "psi_w2", "psi_b2",
              "psi_w3", "psi_b3", "rho_attn_w", "rho_attn_b", "W_k", "rho_w1", "rho_b1",
              "rho_w2", "rho_b2", "rho_w3", "rho_b3"]

    in_maps = []
    for core in range(NCORES):
        rows = [order[s * NCORES + core] for s in range(NSLOTS)]
        m = {
            "times_r": times[rows],
            "values_r": values[rows],
            "meas_r": meas[rows],
            "demo_r": _to_bf16(demo[rows]),
            "lens_bcast": np.ascontiguousarray(
                np.broadcast_to(lengths[rows].astype(np.float32)[None, :], (128, NSLOTS))),
        }
        m.update(const_map)
        for nm in wnames:
            w = inp[nm].astype(np.float32)
            m[nm] = _to_bf16(w) if nm in BF16_W else w
        in_maps.append(m)

    trace = os.environ.get("KERNEL_TRACE", "0") == "1"
    kw = {}
    if trace:
        _maybe_install_ntff_shim()
        kw = dict(trace=True, tmpdir=os.environ.get("KERNEL_TRACE_DIR") or None)
    res = bass_utils.run_bass_kernel_spmd(nc, in_maps, core_ids=list(range(NCORES)), **kw)
    global last_results
    last_results = res
    out = np.zeros((B, 1), np.float32)
    for core in range(NCORES):
        for s in range(NSLOTS):
            out[order[s * NCORES + core], 0] = res.results[core]["out"][s, 0]
    return out

